# revision 56
# baseline (speedup 1.0000x reference)
"""Trainium2 Bass kernel for DeepGraphGO-style 2-layer GraphConv model.

Math (biases are all zero, convs have no nonlinearity between them):
  x1   = relu(features @ W1) * deg_out1^-1/2                     [N, 1024]
  agg1 = (A1 x1) * (deg_in1^-1/2 * deg_out2^-1/2)                [N, 1024]
  agg2 = (A2 agg1) * deg_in2^-1/2                                [N, 1024]
  out  = sigmoid(agg2 @ (Wc1 @ Wc2 @ W2))                        [N, 5000]

The three weight matrices collapse into one host-precomputed Wf [1024, 5000].

Sharding: nodes padded to 20480, 2560 per core (20 blocks of 128).  Each
layer's messages are AllGathered in fp8; each core gathers per-edge source
rows (1 KB fp8) from its local copy and segment-sums them per 128-node dst
block with one-hot fp8 DoubleRow matmuls on the tensor engine.  GEMM1 runs
with the feature tile stationary so x1 is produced node-major (no transposes
needed until the final GEMM).  All matmuls are fp8 e4m3 with f32 PSUM
accumulation; W1 is pre-scaled x8 and Wf x32 to avoid e4m3 subnormals, with
the inverse scales folded into the relu/sigmoid activation scales.
"""

import math
import os
from dataclasses import dataclass

import numpy as np
import ml_dtypes

import concourse.bass as bass
import concourse.bacc as bacc
import concourse.tile as tile
from concourse import mybir
from concourse.masks import make_identity
from concourse.bass_utils import run_bass_kernel_spmd
from concourse.tile_rust import add_dep_helper

FP8 = ml_dtypes.float8_e4m3
BF16 = ml_dtypes.bfloat16
P = 128
W1_SCALE = 8.0
WF_SCALE = 32.0
GO_PAD = 5120


@dataclass(frozen=True)
class Cfg:
    n_nodes: int = 20000          # real nodes
    n_cores: int = 8
    nb: int = 20                  # 128-node blocks per core
    fin: int = 2048               # input feature dim
    h: int = 1024                 # hidden dim
    go: int = 5000                # output dim

    @property
    def npc(self):                # nodes per core (padded)
        return self.nb * P

    @property
    def n_pad(self):
        return self.n_cores * self.npc

    @property
    def ki(self):                 # fin 128-chunks
        return self.fin // P

    @property
    def kh(self):                 # h 128-chunks
        return self.h // P


FULL = Cfg()


# ---------------------------------------------------------------- host prep

def _fp8(x):
    return np.clip(x, -240.0, 240.0).astype(FP8)


def _tile_kmaj(w, k_chunks, ncols):
    """[k_chunks*128, ncols] -> [128, k_chunks*ncols] with dev[p, k*ncols+j] = w[k*128+p, j]."""
    return np.ascontiguousarray(
        w.reshape(k_chunks, P, ncols).transpose(1, 0, 2).reshape(P, k_chunks * ncols)
    )


def _edge_prep(cfg, src, dst, cpb=None):
    """Per-core edge structures for one conv layer.

    Returns (cpb, per_core list of (idx_dev int16 [128, nb*cpb*8],
    wsel_dev fp8 [128, nb*cpb*128])).
    """
    npc, nb = cfg.npc, cfg.nb
    per_core_edges = []
    max_cnt = 0
    for c in range(cfg.n_cores):
        sel = (dst >= c * npc) & (dst < (c + 1) * npc)
        s_e = src[sel].astype(np.int64)
        d_e = (dst[sel] - c * npc).astype(np.int64)
        order = np.argsort(d_e, kind="stable")
        s_e, d_e = s_e[order], d_e[order]
        blk = d_e // P
        counts = np.bincount(blk, minlength=nb)
        max_cnt = max(max_cnt, int(counts.max()))
        per_core_edges.append((s_e, d_e, blk, counts))
    need_cpb = 2 * math.ceil(max_cnt / (2 * P))   # even for DoubleRow pairing
    if cpb is None:
        cpb = need_cpb
    assert cpb >= need_cpb and cpb % 2 == 0
    npad = cpb * P

    out = []
    for s_e, d_e, blk, counts in per_core_edges:
        starts = np.zeros(nb + 1, np.int64)
        np.cumsum(counts, out=starts[1:])
        idx_flat = np.zeros((nb, npad), np.int64)        # gather row ids (0 pad)
        wsel = np.zeros((nb, npad, P), np.float32)       # edge multiplicity per (slot, dst)
        cnts = np.zeros(nb, np.int32)                    # used slots, 128-rounded
        for b in range(nb):
            cnt = int(counts[b])
            if cnt == 0:
                cnts[b] = P
                continue
            sl = slice(starts[b], starts[b + 1])
            # coalesce duplicate (src, dst) pairs into one slot with count weight
            pairs = s_e[sl] * P + (d_e[sl] - b * P)
            uniq, mult = np.unique(pairs, return_counts=True)
            cu = len(uniq)
            idx_flat[b, :cu] = uniq // P
            wsel[b, np.arange(cu), uniq % P] = mult.astype(np.float32)
            cnts[b] = min(npad, ((cu + P - 1) // P) * P)
        # device wsel layout: [128(edge k), nb*cpb*128] ; dev[k, b, j, m] = wsel[b, j*128+k, m]
        wsel_dev = np.ascontiguousarray(
            wsel.reshape(nb, cpb, P, P).transpose(2, 0, 1, 3).reshape(P, nb * cpb * P)
        ).astype(FP8)
        # idx layout: wrapped into 16 partitions, replicated x8
        x = idx_flat.reshape(nb, cpb * 8, 16).transpose(2, 0, 1).reshape(16, nb * cpb * 8)
        idx_dev = np.ascontiguousarray(np.tile(x, (8, 1))).astype(np.int16)
        out.append((idx_dev, wsel_dev, cnts))
    # static per-block gather chunk count: max over cores (same SPMD program)
    cbs = np.max(np.stack([o[2] for o in out]) // P, axis=0).astype(int)
    return cpb, out, cbs


def prep_inputs(cfg, inputs):
    """Build the SPMD per-core input maps. Returns (cpb, in_maps)."""
    f32 = np.float32
    feats = np.asarray(inputs["features"], f32)
    W1 = np.asarray(inputs["W1"], f32)
    Wc1 = np.asarray(inputs["Wc1"], f32)
    Wc2 = np.asarray(inputs["Wc2"], f32)
    W2 = np.asarray(inputs["W2"], f32)
    for bname in ("b1", "bc1", "bc2", "b2"):
        assert not np.any(np.asarray(inputs[bname])), f"nonzero bias {bname} unsupported"
    src1 = np.asarray(inputs["src1"]).astype(np.int64)
    dst1 = np.asarray(inputs["dst1"]).astype(np.int64)
    src2 = np.asarray(inputs["src2"]).astype(np.int64)
    dst2 = np.asarray(inputs["dst2"]).astype(np.int64)

    npc, nb, n_pad, ki, kh = cfg.npc, cfg.nb, cfg.n_pad, cfg.ki, cfg.kh

    deg_out1 = np.maximum(np.bincount(src1, minlength=n_pad), 1.0).astype(f32) ** -0.5
    deg_in1 = np.maximum(np.bincount(dst1, minlength=n_pad), 1.0).astype(f32) ** -0.5
    deg_out2 = np.maximum(np.bincount(src2, minlength=n_pad), 1.0).astype(f32) ** -0.5
    deg_in2 = np.maximum(np.bincount(dst2, minlength=n_pad), 1.0).astype(f32) ** -0.5

    featp = np.zeros((n_pad, cfg.fin), f32)
    featp[: cfg.n_nodes] = feats

    # fused output weight, x32 to dodge e4m3 subnormals (undone at the sigmoid)
    Wf = (Wc1 @ Wc2) @ W2
    Wfp = np.zeros((cfg.h, GO_PAD), f32)
    Wfp[:, : cfg.go] = Wf * WF_SCALE
    wf_dev = _fp8(_tile_kmaj(Wfp, kh, GO_PAD))
    w1_dev = _fp8(_tile_kmaj(W1 * W1_SCALE, ki, cfg.h))

    cpb1, e1, cbs1 = _edge_prep(cfg, src1, dst1)
    cpb2, e2, cbs2 = _edge_prep(cfg, src2, dst2)
    cpb = max(cpb1, cpb2)
    if cpb1 < cpb:
        _, e1, cbs1 = _edge_prep(cfg, src1, dst1, cpb)
    if cpb2 < cpb:
        _, e2, cbs2 = _edge_prep(cfg, src2, dst2, cpb)

    in_maps = []
    for c in range(cfg.n_cores):
        lo, hi = c * npc, (c + 1) * npc
        # block-major stationary feature tiles: dev[p, b, k, n] = feat[lo+b*128+n, k*128+p]
        a = featp[lo:hi].reshape(nb, P, ki, P)           # [b, n, k, p]
        featT_dev = _fp8(np.ascontiguousarray(a.transpose(3, 0, 2, 1).reshape(P, nb * ki * P)))
        s1 = (deg_out1[lo:hi] / W1_SCALE).reshape(nb, P).T         # relu scale
        s2 = (deg_in1[lo:hi] * deg_out2[lo:hi]).reshape(nb, P).T   # conv1 out scale
        s3 = deg_in2[lo:hi].reshape(nb, P).T                       # conv2 out scale
        s_all = np.ascontiguousarray(np.concatenate([s1, s2, s3], axis=1)).astype(f32)
        in_maps.append(
            {
                "featT": featT_dev,
                "w1": w1_dev,
                "wf": wf_dev,
                "s_all": s_all,
                "idx1": e1[c][0],
                "wsel1": e1[c][1],
                "idx2": e2[c][0],
                "wsel2": e2[c][1],
                "gcnt": np.concatenate([e1[c][2], e2[c][2]]).reshape(1, 2 * nb),
            }
        )
    return cpb, in_maps, (list(cbs1), list(cbs2))


# ---------------------------------------------------------------- device build

def build_bass(cfg, cpb, cbs=None):
    f32, fp8, bf16, i16 = mybir.dt.float32, mybir.dt.float8e4, mybir.dt.bfloat16, mybir.dt.int16
    nb, npc, ki, kh, h, go = cfg.nb, cfg.npc, cfg.ki, cfg.kh, cfg.h, cfg.go
    DR = mybir.MatmulPerfMode.DoubleRow

    if cbs is None:
        cbs = ([cpb] * cfg.nb, [cpb] * cfg.nb)
    nc = bacc.Bacc(
        "TRN2", target_bir_lowering=False, debug=False, num_devices=cfg.n_cores,
        dynamic_dma_scratch_size=32768,
    )

    featT = nc.dram_tensor("featT", [P, nb * ki * P], fp8, kind="ExternalInput")
    w1 = nc.dram_tensor("w1", [P, ki * h], fp8, kind="ExternalInput")
    wf = nc.dram_tensor("wf", [P, kh * GO_PAD], fp8, kind="ExternalInput")
    s_all = nc.dram_tensor("s_all", [P, 3 * nb], f32, kind="ExternalInput")
    idx1 = nc.dram_tensor("idx1", [P, nb * cpb * 8], i16, kind="ExternalInput")
    wsel1 = nc.dram_tensor("wsel1", [P, nb * cpb * P], fp8, kind="ExternalInput")
    idx2 = nc.dram_tensor("idx2", [P, nb * cpb * 8], i16, kind="ExternalInput")
    wsel2 = nc.dram_tensor("wsel2", [P, nb * cpb * P], fp8, kind="ExternalInput")
    gcnt = nc.dram_tensor("gcnt", [1, 2 * nb], mybir.dt.int32, kind="ExternalInput")
    out_d = nc.dram_tensor("out", [npc, go], bf16, kind="ExternalOutput")

    ag1_in = nc.dram_tensor("ag1_in", [npc, h], fp8, kind="Internal")
    ag1_out = nc.dram_tensor("ag1_out", [cfg.n_pad, h], fp8, kind="Internal", addr_space="Shared")
    ag2_in = nc.dram_tensor("ag2_in", [npc, h], fp8, kind="Internal")
    ag2_out = nc.dram_tensor("ag2_out", [cfg.n_pad, h], fp8, kind="Internal", addr_space="Shared")

    rg = [list(range(cfg.n_cores))]
    mult = mybir.AluOpType.mult
    Relu = mybir.ActivationFunctionType.Relu
    Sigmoid = mybir.ActivationFunctionType.Sigmoid

    wsel1_r = wsel1[:].rearrange("p (c n) -> p c n", c=nb * cpb)
    wsel2_r = wsel2[:].rearrange("p (c n) -> p c n", c=nb * cpb)

    with tile.TileContext(nc) as tc:
        with tc.tile_pool(name="consts", bufs=1) as consts, \
             tc.tile_pool(name="wts", bufs=1) as wts, \
             tc.tile_pool(name="ft", bufs=2) as ft_p, \
             tc.tile_pool(name="mmps", bufs=3, space="PSUM") as mm_p, \
             tc.tile_pool(name="rowout", bufs=3) as row_p, \
             tc.tile_pool(name="gat", bufs=3) as gat_p, \
             tc.tile_pool(name="wsl", bufs=3) as wsl_p, \
             tc.tile_pool(name="x3t", bufs=2) as x3t_p, \
             tc.tile_pool(name="gops", bufs=2, space="PSUM") as go_p, \
             tc.tile_pool(name="ob", bufs=3) as ob_p:

            s_sb = consts.tile([P, 3 * nb], f32)
            nc.sync.dma_start(out=s_sb[:], in_=s_all[:])
            idx1_sb = consts.tile([P, nb * cpb * 8], i16)
            nc.sync.dma_start(out=idx1_sb[:], in_=idx1[:])
            idx2_sb = consts.tile([P, nb * cpb * 8], i16)
            nc.sync.dma_start(out=idx2_sb[:], in_=idx2[:])
            ident = consts.tile([P, P], bf16)
            make_identity(nc, ident[:])
            gcnt_sb = consts.tile([1, 2 * nb], mybir.dt.int32)
            nc.sync.dma_start(out=gcnt_sb[:], in_=gcnt[:])
            cnt_reg = nc.alloc_register(mybir.EngineType.Pool, "gcnt_reg")

            wf_sb = wts.tile([P, kh, GO_PAD], fp8)
            nc.sync.dma_start(out=wf_sb[:], in_=wf[:].rearrange("p (k n) -> p k n", k=kh))
            w1_sb = wts.tile([P, ki, h], fp8)
            nc.sync.dma_start(out=w1_sb[:], in_=w1[:].rearrange("p (k n) -> p k n", k=ki))

            # ---------------- phase 1: x1 = relu(feat @ W1) * s1, node-major
            for b in range(nb):
                ft = ft_p.tile([P, ki, P], fp8, tag="ft")
                nc.sync.dma_start(
                    out=ft[:],
                    in_=featT[:, b * ki * P:(b + 1) * ki * P].rearrange("p (k n) -> p k n", k=ki),
                )
                ps = mm_p.tile([P, h], f32, tag="mm")
                for c in range(ki // 2):
                    for hh in range(h // 512):
                        nc.tensor.matmul(
                            ps[:, hh * 512:(hh + 1) * 512],
                            lhsT=ft[:, 2 * c:2 * c + 2, :],
                            rhs=w1_sb[:, 2 * c:2 * c + 2, hh * 512:(hh + 1) * 512],
                            start=(c == 0),
                            stop=(c == ki // 2 - 1),
                            perf_mode=DR,
                        )
                x1t = row_p.tile([P, h], fp8, tag="row")
                nc.scalar.activation(out=x1t[:], in_=ps[:], func=Relu, scale=s_sb[:, b:b + 1])
                nc.sync.dma_start(out=ag1_in[b * P:(b + 1) * P, :], in_=x1t[:])

            ag1_cc = nc.gpsimd.collective_compute(
                "AllGather", mybir.AluOpType.bypass,
                ins=[ag1_in[:]], outs=[ag1_out[:]], replica_groups=rg,
            )

            # -------- conv machinery: descriptor pre-generation + triggered fire.
            # Preps (descgen on Q7) have only the idx tensor as a sync dep, so
            # the scheduler can run them under GEMM1/collectives; the data RAW
            # on ag*_out and the gt-slot WAR defer to the trigger.  3-deep
            # buffer pipeline, one gather call per 128-dst-node block, padded
            # idx slots are -1 (descgen/DMA skip them; stale gt bytes are
            # zeroed once below and otherwise finite, and wsel is 0 there).
            DEPTH = 2
            for _ in range(3):               # every gat_p slot, matches bufs=3
                t = gat_p.tile([P, cpb, h], fp8, tag="gt")
                nc.vector.memset(t[:], 0.0)

            def conv(ag_out_t, idx_sb, wsel_r, q, sem, ag_inst, out_cb, conv_cbs):
                tiles = {}

                def prep(b):
                    gt = gat_p.tile([P, cpb, h], fp8, tag="gt")
                    tiles[b] = gt
                    # one call per block, static per-block row count (max
                    # over cores) so descgen skips full-pad chunks; >64
                    # descs/engine needs multi-packet.  (A runtime-register
                    # count via num_idxs_reg=reg crashes the runtime, as does
                    # -1 idx padding; prepare_only/trigger pre-generation is
                    # correct with explicit edges but measured slower.)
                    cb = conv_cbs[b]
                    nc.gpsimd.dma_gather(
                        gt[:, :cb, :], ag_out_t[:],
                        idx_sb[:, b * cpb * 8:b * cpb * 8 + cb * 8],
                        cb * P, cb * P, h,
                        single_packet=False, queue_num=q,
                    )

                for b in range(nb):
                    prep(b)
                    gt = tiles.pop(b)
                    ws = wsl_p.tile([P, cpb, P], fp8, tag="ws")
                    nc.sync.dma_start(out=ws[:], in_=wsel_r[:, b * cpb:(b + 1) * cpb, :])
                    ps = mm_p.tile([P, h], f32, tag="mm")
                    for hh in range(h // 512):
                        for j in range(cpb // 2):
                            nc.tensor.matmul(
                                ps[:, hh * 512:(hh + 1) * 512],
                                lhsT=ws[:, 2 * j:2 * j + 2, :],
                                rhs=gt[:, 2 * j:2 * j + 2, hh * 512:(hh + 1) * 512],
                                start=(j == 0),
                                stop=(j == cpb // 2 - 1),
                                perf_mode=DR,
                            )
                    out_cb(b, ps)

            def conv1_out(b, ps):
                aggt = row_p.tile([P, h], fp8, tag="row")
                nc.vector.tensor_scalar(
                    out=aggt[:], in0=ps[:], scalar1=s_sb[:, nb + b:nb + b + 1],
                    scalar2=None, op0=mult,
                )
                nc.sync.dma_start(out=ag2_in[b * P:(b + 1) * P, :], in_=aggt[:])

            dma_sem1 = nc.alloc_semaphore("gsem1")
            conv(ag1_out, idx1_sb, wsel1_r, 0, dma_sem1, ag1_cc, conv1_out, cbs[0])

            ag2_cc = nc.gpsimd.collective_compute(
                "AllGather", mybir.AluOpType.bypass,
                ins=[ag2_in[:]], outs=[ag2_out[:]], replica_groups=rg,
            )

            # ---------------- phase 3: agg2 = (A2 agg1) * s3 ; out = sigmoid(agg2 @ Wf / 32)
            def conv2_out(b, ps):
                x3 = row_p.tile([P, h], bf16, tag="rowb")
                nc.vector.tensor_scalar(
                    out=x3[:], in0=ps[:], scalar1=s_sb[:, 2 * nb + b:2 * nb + b + 1],
                    scalar2=None, op0=mult,
                )
                # transpose on the Sync engine's x-bar (PE queue is saturated),
                # then one fused bf16->fp8 cast on DVE
                x3tb = x3t_p.tile([P, kh, P], bf16, tag="x3tb")
                for m in range(kh):
                    nc.sync.dma_start_transpose(
                        out=x3tb[:, m, :], in_=x3[:, m * P:(m + 1) * P]
                    )
                x3t = x3t_p.tile([P, kh, P], fp8, tag="x3t")
                nc.vector.tensor_copy(out=x3t[:], in_=x3tb[:])
                for g in range(GO_PAD // 512):
                    gn = min(512, go - g * 512)
                    if gn <= 0:
                        break
                    gp = go_p.tile([P, 512], f32, tag="gp")
                    for c in range(kh // 2):
                        nc.tensor.matmul(
                            gp[:],
                            lhsT=x3t[:, 2 * c:2 * c + 2, :],
                            rhs=wf_sb[:, 2 * c:2 * c + 2, g * 512:(g + 1) * 512],
                            start=(c == 0),
                            stop=(c == kh // 2 - 1),
                            perf_mode=DR,
                        )
                    o = ob_p.tile([P, 512], bf16, tag="ob")
                    nc.scalar.activation(
                        out=o[:, :gn], in_=gp[:, :gn], func=Sigmoid, scale=1.0 / WF_SCALE,
                    )
                    nc.sync.dma_start(
                        out=out_d[b * P:(b + 1) * P, g * 512:g * 512 + gn], in_=o[:, :gn]
                    )

            dma_sem2 = nc.alloc_semaphore("gsem2")
            conv(ag2_out, idx2_sb, wsel2_r, 0, dma_sem2, ag2_cc, conv2_out, cbs[1])

    nc.compile()
    return nc


# ---------------------------------------------------------------- entry point

def _run_hw(cfg, inputs, trace=False):
    cpb, in_maps, cbs = prep_inputs(cfg, inputs)
    nc = build_bass(cfg, cpb, cbs)
    res = run_bass_kernel_spmd(nc, in_maps, core_ids=list(range(cfg.n_cores)), trace=trace)
    full = np.concatenate([res.results[c]["out"] for c in range(cfg.n_cores)], axis=0)
    return full[: cfg.n_nodes].astype(np.float32), res


def kernel(**inputs) -> np.ndarray:
    trace = bool(int(os.environ.get("GNN_TRACE", "0")))
    out, res = _run_hw(FULL, inputs, trace=trace)
    if trace and res.exec_time_ns is not None:
        print(f"HW exec time: {res.exec_time_ns} ns")
    return out


# revision 59
# speedup vs baseline: 1.3191x; 1.3191x over previous
"""Trainium2 Bass kernel for DeepGraphGO-style 2-layer GraphConv model.

Math (biases are all zero, convs have no nonlinearity between them):
  x1   = relu(features @ W1) * deg_out1^-1/2                     [N, 1024]
  agg1 = (A1 x1) * (deg_in1^-1/2 * deg_out2^-1/2)                [N, 1024]
  agg2 = (A2 agg1) * deg_in2^-1/2                                [N, 1024]
  out  = sigmoid(agg2 @ (Wc1 @ Wc2 @ W2))                        [N, 5000]

The three weight matrices collapse into one host-precomputed Wf [1024, 5000].

Sharding: nodes padded to 20480, 2560 per core (20 blocks of 128).  Each
layer's messages are AllGathered in fp8; each core gathers per-edge source
rows (1 KB fp8) from its local copy and segment-sums them per 128-node dst
block with one-hot fp8 DoubleRow matmuls on the tensor engine.  GEMM1 runs
with the feature tile stationary so x1 is produced node-major (no transposes
needed until the final GEMM).  All matmuls are fp8 e4m3 with f32 PSUM
accumulation; W1 is pre-scaled x8 and Wf x32 to avoid e4m3 subnormals, with
the inverse scales folded into the relu/sigmoid activation scales.
"""

import math
import os
from dataclasses import dataclass

import numpy as np
import ml_dtypes

import concourse.bass as bass
import concourse.bacc as bacc
import concourse.tile as tile
from concourse import mybir
from concourse.masks import make_identity
from concourse.bass_utils import run_bass_kernel_spmd
from concourse.tile_rust import add_dep_helper

FP8 = ml_dtypes.float8_e4m3
BF16 = ml_dtypes.bfloat16
P = 128
W1_SCALE = 8.0
WF_SCALE = 32.0
GO_PAD = 5120


@dataclass(frozen=True)
class Cfg:
    n_nodes: int = 20000          # real nodes
    n_cores: int = 8
    nb: int = 20                  # 128-node blocks per core
    fin: int = 2048               # input feature dim
    h: int = 1024                 # hidden dim
    go: int = 5000                # output dim

    @property
    def npc(self):                # nodes per core (padded)
        return self.nb * P

    @property
    def n_pad(self):
        return self.n_cores * self.npc

    @property
    def ki(self):                 # fin 128-chunks
        return self.fin // P

    @property
    def kh(self):                 # h 128-chunks
        return self.h // P


FULL = Cfg()


# ---------------------------------------------------------------- host prep

def _fp8(x):
    return np.clip(x, -240.0, 240.0).astype(FP8)


def _tile_kmaj(w, k_chunks, ncols):
    """[k_chunks*128, ncols] -> [128, k_chunks*ncols] with dev[p, k*ncols+j] = w[k*128+p, j]."""
    return np.ascontiguousarray(
        w.reshape(k_chunks, P, ncols).transpose(1, 0, 2).reshape(P, k_chunks * ncols)
    )


def _edge_prep(cfg, src, dst, cpb=None):
    """Per-core edge structures for one conv layer.

    Returns (cpb, per_core list of (idx_dev int16 [128, nb*cpb*8],
    wsel_dev fp8 [128, nb*cpb*128])).
    """
    npc, nb = cfg.npc, cfg.nb
    per_core_edges = []
    max_cnt = 0
    for c in range(cfg.n_cores):
        sel = (dst >= c * npc) & (dst < (c + 1) * npc)
        s_e = src[sel].astype(np.int64)
        d_e = (dst[sel] - c * npc).astype(np.int64)
        order = np.argsort(d_e, kind="stable")
        s_e, d_e = s_e[order], d_e[order]
        blk = d_e // P
        counts = np.bincount(blk, minlength=nb)
        max_cnt = max(max_cnt, int(counts.max()))
        per_core_edges.append((s_e, d_e, blk, counts))
    need_cpb = 2 * math.ceil(max_cnt / (2 * P))   # even for DoubleRow pairing
    if cpb is None:
        cpb = need_cpb
    assert cpb >= need_cpb and cpb % 2 == 0
    npad = cpb * P

    out = []
    for s_e, d_e, blk, counts in per_core_edges:
        starts = np.zeros(nb + 1, np.int64)
        np.cumsum(counts, out=starts[1:])
        idx_flat = np.zeros((nb, npad), np.int64)        # gather row ids (0 pad)
        wsel = np.zeros((nb, npad, P), np.float32)       # edge multiplicity per (slot, dst)
        cnts = np.zeros(nb, np.int32)                    # used slots, 128-rounded
        for b in range(nb):
            cnt = int(counts[b])
            if cnt == 0:
                cnts[b] = P
                continue
            sl = slice(starts[b], starts[b + 1])
            # coalesce duplicate (src, dst) pairs into one slot with count weight
            pairs = s_e[sl] * P + (d_e[sl] - b * P)
            uniq, mult = np.unique(pairs, return_counts=True)
            cu = len(uniq)
            idx_flat[b, :cu] = uniq // P
            wsel[b, np.arange(cu), uniq % P] = mult.astype(np.float32)
            cnts[b] = min(npad, ((cu + P - 1) // P) * P)
        # device wsel layout: [128(edge k), nb*cpb*128] ; dev[k, b, j, m] = wsel[b, j*128+k, m]
        wsel_dev = np.ascontiguousarray(
            wsel.reshape(nb, cpb, P, P).transpose(2, 0, 1, 3).reshape(P, nb * cpb * P)
        ).astype(FP8)
        # idx layout: wrapped into 16 partitions, replicated x8
        x = idx_flat.reshape(nb, cpb * 8, 16).transpose(2, 0, 1).reshape(16, nb * cpb * 8)
        idx_dev = np.ascontiguousarray(np.tile(x, (8, 1))).astype(np.int16)
        out.append((idx_dev, wsel_dev, cnts))
    # static per-block gather chunk count: max over cores (same SPMD program)
    cbs = np.max(np.stack([o[2] for o in out]) // P, axis=0).astype(int)
    return cpb, out, cbs


def prep_inputs(cfg, inputs):
    """Build the SPMD per-core input maps. Returns (cpb, in_maps)."""
    f32 = np.float32
    feats = np.asarray(inputs["features"], f32)
    W1 = np.asarray(inputs["W1"], f32)
    Wc1 = np.asarray(inputs["Wc1"], f32)
    Wc2 = np.asarray(inputs["Wc2"], f32)
    W2 = np.asarray(inputs["W2"], f32)
    for bname in ("b1", "bc1", "bc2", "b2"):
        assert not np.any(np.asarray(inputs[bname])), f"nonzero bias {bname} unsupported"
    src1 = np.asarray(inputs["src1"]).astype(np.int64)
    dst1 = np.asarray(inputs["dst1"]).astype(np.int64)
    src2 = np.asarray(inputs["src2"]).astype(np.int64)
    dst2 = np.asarray(inputs["dst2"]).astype(np.int64)

    npc, nb, n_pad, ki, kh = cfg.npc, cfg.nb, cfg.n_pad, cfg.ki, cfg.kh

    deg_out1 = np.maximum(np.bincount(src1, minlength=n_pad), 1.0).astype(f32) ** -0.5
    deg_in1 = np.maximum(np.bincount(dst1, minlength=n_pad), 1.0).astype(f32) ** -0.5
    deg_out2 = np.maximum(np.bincount(src2, minlength=n_pad), 1.0).astype(f32) ** -0.5
    deg_in2 = np.maximum(np.bincount(dst2, minlength=n_pad), 1.0).astype(f32) ** -0.5

    featp = np.zeros((n_pad, cfg.fin), f32)
    featp[: cfg.n_nodes] = feats

    # fused output weight, x32 to dodge e4m3 subnormals (undone at the sigmoid)
    Wf = (Wc1 @ Wc2) @ W2
    Wfp = np.zeros((cfg.h, GO_PAD), f32)
    Wfp[:, : cfg.go] = Wf * WF_SCALE
    wf_dev = _fp8(_tile_kmaj(Wfp, kh, GO_PAD))
    w1_dev = _fp8(_tile_kmaj(W1 * W1_SCALE, ki, cfg.h))

    cpb1, e1, cbs1 = _edge_prep(cfg, src1, dst1)
    cpb2, e2, cbs2 = _edge_prep(cfg, src2, dst2)
    cpb = max(cpb1, cpb2)
    if cpb1 < cpb:
        _, e1, cbs1 = _edge_prep(cfg, src1, dst1, cpb)
    if cpb2 < cpb:
        _, e2, cbs2 = _edge_prep(cfg, src2, dst2, cpb)

    in_maps = []
    for c in range(cfg.n_cores):
        lo, hi = c * npc, (c + 1) * npc
        # block-major stationary feature tiles: dev[p, b, k, n] = feat[lo+b*128+n, k*128+p]
        a = featp[lo:hi].reshape(nb, P, ki, P)           # [b, n, k, p]
        featT_dev = _fp8(np.ascontiguousarray(a.transpose(3, 0, 2, 1).reshape(P, nb * ki * P)))
        s1 = (deg_out1[lo:hi] / W1_SCALE).reshape(nb, P).T         # relu scale
        s2 = (deg_in1[lo:hi] * deg_out2[lo:hi]).reshape(nb, P).T   # conv1 out scale
        s3 = deg_in2[lo:hi].reshape(nb, P).T                       # conv2 out scale
        s_all = np.ascontiguousarray(np.concatenate([s1, s2, s3], axis=1)).astype(f32)
        in_maps.append(
            {
                "featT": featT_dev,
                "w1": w1_dev,
                "wf": wf_dev,
                "s_all": s_all,
                "idx1": e1[c][0],
                "wsel1": e1[c][1],
                "idx2": e2[c][0],
                "wsel2": e2[c][1],
                "gcnt": np.concatenate([e1[c][2], e2[c][2]]).reshape(1, 2 * nb),
            }
        )
    return cpb, in_maps, (list(cbs1), list(cbs2))


# ---------------------------------------------------------------- device build

def build_bass(cfg, cpb, cbs=None):
    f32, fp8, bf16, i16 = mybir.dt.float32, mybir.dt.float8e4, mybir.dt.bfloat16, mybir.dt.int16
    nb, npc, ki, kh, h, go = cfg.nb, cfg.npc, cfg.ki, cfg.kh, cfg.h, cfg.go
    DR = mybir.MatmulPerfMode.DoubleRow

    if cbs is None:
        cbs = ([cpb] * cfg.nb, [cpb] * cfg.nb)
    nc = bacc.Bacc(
        "TRN2", target_bir_lowering=False, debug=False, num_devices=cfg.n_cores,
        dynamic_dma_scratch_size=32768,
    )

    featT = nc.dram_tensor("featT", [P, nb * ki * P], fp8, kind="ExternalInput")
    w1 = nc.dram_tensor("w1", [P, ki * h], fp8, kind="ExternalInput")
    wf = nc.dram_tensor("wf", [P, kh * GO_PAD], fp8, kind="ExternalInput")
    s_all = nc.dram_tensor("s_all", [P, 3 * nb], f32, kind="ExternalInput")
    idx1 = nc.dram_tensor("idx1", [P, nb * cpb * 8], i16, kind="ExternalInput")
    wsel1 = nc.dram_tensor("wsel1", [P, nb * cpb * P], fp8, kind="ExternalInput")
    idx2 = nc.dram_tensor("idx2", [P, nb * cpb * 8], i16, kind="ExternalInput")
    wsel2 = nc.dram_tensor("wsel2", [P, nb * cpb * P], fp8, kind="ExternalInput")
    gcnt = nc.dram_tensor("gcnt", [1, 2 * nb], mybir.dt.int32, kind="ExternalInput")
    out_d = nc.dram_tensor("out", [npc, go], bf16, kind="ExternalOutput")

    ag1_in = nc.dram_tensor("ag1_in", [npc, h], fp8, kind="Internal")
    ag1_out = nc.dram_tensor("ag1_out", [cfg.n_pad, h], fp8, kind="Internal", addr_space="Shared")
    ag2_in = nc.dram_tensor("ag2_in", [npc, h], fp8, kind="Internal")
    ag2_out = nc.dram_tensor("ag2_out", [cfg.n_pad, h], fp8, kind="Internal", addr_space="Shared")

    rg = [list(range(cfg.n_cores))]
    mult = mybir.AluOpType.mult
    Relu = mybir.ActivationFunctionType.Relu
    Sigmoid = mybir.ActivationFunctionType.Sigmoid

    wsel1_r = wsel1[:].rearrange("p (c n) -> p c n", c=nb * cpb)
    wsel2_r = wsel2[:].rearrange("p (c n) -> p c n", c=nb * cpb)

    with tile.TileContext(nc) as tc:
        with tc.tile_pool(name="consts", bufs=1) as consts, \
             tc.tile_pool(name="wts", bufs=1) as wts, \
             tc.tile_pool(name="ft", bufs=2) as ft_p, \
             tc.tile_pool(name="mmps", bufs=2, space="PSUM") as mm_p, \
             tc.tile_pool(name="rowout", bufs=3) as row_p, \
             tc.tile_pool(name="gat", bufs=3) as gat_p, \
             tc.tile_pool(name="wsl", bufs=3) as wsl_p, \
             tc.tile_pool(name="tps", bufs=2, space="PSUM") as tps_p, \
             tc.tile_pool(name="x3t", bufs=2) as x3t_p, \
             tc.tile_pool(name="gops", bufs=2, space="PSUM") as go_p, \
             tc.tile_pool(name="ob", bufs=3) as ob_p:

            s_sb = consts.tile([P, 3 * nb], f32)
            nc.sync.dma_start(out=s_sb[:], in_=s_all[:])
            idx1_sb = consts.tile([P, nb * cpb * 8], i16)
            nc.sync.dma_start(out=idx1_sb[:], in_=idx1[:])
            idx2_sb = consts.tile([P, nb * cpb * 8], i16)
            nc.sync.dma_start(out=idx2_sb[:], in_=idx2[:])
            ident = consts.tile([P, P], bf16)
            make_identity(nc, ident[:])
            gcnt_sb = consts.tile([1, 2 * nb], mybir.dt.int32)
            nc.sync.dma_start(out=gcnt_sb[:], in_=gcnt[:])
            cnt_reg = nc.alloc_register(mybir.EngineType.Pool, "gcnt_reg")

            wf_sb = wts.tile([P, kh, GO_PAD], fp8)
            nc.sync.dma_start(out=wf_sb[:], in_=wf[:].rearrange("p (k n) -> p k n", k=kh))
            w1_sb = wts.tile([P, ki, h], fp8)
            nc.sync.dma_start(out=w1_sb[:], in_=w1[:].rearrange("p (k n) -> p k n", k=ki))

            # ---------------- phase 1: x1 = relu(feat @ W1) * s1, node-major
            for b in range(nb):
                ft = ft_p.tile([P, ki, P], fp8, tag="ft")
                nc.sync.dma_start(
                    out=ft[:],
                    in_=featT[:, b * ki * P:(b + 1) * ki * P].rearrange("p (k n) -> p k n", k=ki),
                )
                ps = mm_p.tile([P, h], f32, tag="mm")
                for c in range(ki // 2):
                    for hh in range(h // 512):
                        nc.tensor.matmul(
                            ps[:, hh * 512:(hh + 1) * 512],
                            lhsT=ft[:, 2 * c:2 * c + 2, :],
                            rhs=w1_sb[:, 2 * c:2 * c + 2, hh * 512:(hh + 1) * 512],
                            start=(c == 0),
                            stop=(c == ki // 2 - 1),
                            perf_mode=DR,
                        )
                x1t = row_p.tile([P, h], fp8, tag="row")
                nc.scalar.activation(out=x1t[:], in_=ps[:], func=Relu, scale=s_sb[:, b:b + 1])
                nc.sync.dma_start(out=ag1_in[b * P:(b + 1) * P, :], in_=x1t[:])

            ag1_cc = nc.gpsimd.collective_compute(
                "AllGather", mybir.AluOpType.bypass,
                ins=[ag1_in[:]], outs=[ag1_out[:]], replica_groups=rg,
            )

            # -------- conv machinery: descriptor pre-generation + triggered fire.
            # Preps (descgen on Q7) have only the idx tensor as a sync dep, so
            # the scheduler can run them under GEMM1/collectives; the data RAW
            # on ag*_out and the gt-slot WAR defer to the trigger.  3-deep
            # buffer pipeline, one gather call per 128-dst-node block, padded
            # idx slots are -1 (descgen/DMA skip them; stale gt bytes are
            # zeroed once below and otherwise finite, and wsel is 0 there).
            DEPTH = 2
            for _ in range(3):               # every gat_p slot, matches bufs=3
                t = gat_p.tile([P, cpb, h], fp8, tag="gt")
                nc.vector.memset(t[:], 0.0)

            def conv(ag_out_t, idx_sb, wsel_r, q, sem, ag_inst, out_cb, conv_cbs):
                tiles = {}

                def prep(b):
                    gt = gat_p.tile([P, cpb, h], fp8, tag="gt")
                    tiles[b] = gt
                    # one call per block, static per-block row count (max
                    # over cores) so descgen skips full-pad chunks; >64
                    # descs/engine needs multi-packet.  (A runtime-register
                    # count via num_idxs_reg=reg crashes the runtime, as does
                    # -1 idx padding; prepare_only/trigger pre-generation is
                    # correct with explicit edges but measured slower.)
                    cb = conv_cbs[b]
                    nc.gpsimd.dma_gather(
                        gt[:, :cb, :], ag_out_t[:],
                        idx_sb[:, b * cpb * 8:b * cpb * 8 + cb * 8],
                        cb * P, cb * P, h,
                        single_packet=False, queue_num=q,
                    )

                for b in range(nb):
                    prep(b)
                    gt = tiles.pop(b)
                    ws = wsl_p.tile([P, cpb, P], fp8, tag="ws")
                    nc.sync.dma_start(out=ws[:], in_=wsel_r[:, b * cpb:(b + 1) * cpb, :])
                    ps = mm_p.tile([P, h], f32, tag="mm")
                    for hh in range(h // 512):
                        for j in range(cpb // 2):
                            nc.tensor.matmul(
                                ps[:, hh * 512:(hh + 1) * 512],
                                lhsT=ws[:, 2 * j:2 * j + 2, :],
                                rhs=gt[:, 2 * j:2 * j + 2, hh * 512:(hh + 1) * 512],
                                start=(j == 0),
                                stop=(j == cpb // 2 - 1),
                                perf_mode=DR,
                            )
                    out_cb(b, ps)

            def conv1_out(b, ps):
                aggt = row_p.tile([P, h], fp8, tag="row")
                nc.vector.tensor_scalar(
                    out=aggt[:], in0=ps[:], scalar1=s_sb[:, nb + b:nb + b + 1],
                    scalar2=None, op0=mult,
                )
                nc.sync.dma_start(out=ag2_in[b * P:(b + 1) * P, :], in_=aggt[:])

            dma_sem1 = nc.alloc_semaphore("gsem1")
            conv(ag1_out, idx1_sb, wsel1_r, 0, dma_sem1, ag1_cc, conv1_out, cbs[0])

            ag2_cc = nc.gpsimd.collective_compute(
                "AllGather", mybir.AluOpType.bypass,
                ins=[ag2_in[:]], outs=[ag2_out[:]], replica_groups=rg,
            )

            # ---------------- phase 3: agg2 = (A2 agg1) * s3 ; out = sigmoid(agg2 @ Wf / 32)
            def conv2_out(b, ps):
                x3 = row_p.tile([P, h], bf16, tag="rowb")
                nc.vector.tensor_scalar(
                    out=x3[:], in0=ps[:], scalar1=s_sb[:, 2 * nb + b:2 * nb + b + 1],
                    scalar2=None, op0=mult,
                )
                x3t = x3t_p.tile([P, kh, P], fp8, tag="x3t")
                for m in range(kh):
                    tp = tps_p.tile([P, P], bf16, tag="tp")
                    nc.tensor.transpose(out=tp[:], in_=x3[:, m * P:(m + 1) * P], identity=ident[:])
                    nc.vector.tensor_copy(out=x3t[:, m, :], in_=tp[:])
                for g in range(GO_PAD // 512):
                    gn = min(512, go - g * 512)
                    if gn <= 0:
                        break
                    gp = go_p.tile([P, 512], f32, tag="gp")
                    for c in range(kh // 2):
                        nc.tensor.matmul(
                            gp[:],
                            lhsT=x3t[:, 2 * c:2 * c + 2, :],
                            rhs=wf_sb[:, 2 * c:2 * c + 2, g * 512:(g + 1) * 512],
                            start=(c == 0),
                            stop=(c == kh // 2 - 1),
                            perf_mode=DR,
                        )
                    o = ob_p.tile([P, 512], bf16, tag="ob")
                    nc.scalar.activation(
                        out=o[:, :gn], in_=gp[:, :gn], func=Sigmoid, scale=1.0 / WF_SCALE,
                    )
                    nc.sync.dma_start(
                        out=out_d[b * P:(b + 1) * P, g * 512:g * 512 + gn], in_=o[:, :gn]
                    )

            dma_sem2 = nc.alloc_semaphore("gsem2")
            conv(ag2_out, idx2_sb, wsel2_r, 0, dma_sem2, ag2_cc, conv2_out, cbs[1])

    nc.compile()
    return nc


# ---------------------------------------------------------------- entry point

def _run_hw(cfg, inputs, trace=False):
    cpb, in_maps, cbs = prep_inputs(cfg, inputs)
    nc = build_bass(cfg, cpb, cbs)
    res = run_bass_kernel_spmd(nc, in_maps, core_ids=list(range(cfg.n_cores)), trace=trace)
    full = np.concatenate([res.results[c]["out"] for c in range(cfg.n_cores)], axis=0)
    return full[: cfg.n_nodes].astype(np.float32), res


def kernel(**inputs) -> np.ndarray:
    trace = bool(int(os.environ.get("GNN_TRACE", "0")))
    out, res = _run_hw(FULL, inputs, trace=trace)
    if trace and res.exec_time_ns is not None:
        print(f"HW exec time: {res.exec_time_ns} ns")
    return out
